# revision 24
# baseline (speedup 1.0000x reference)
"""DCRNCognition Trainium2 kernel (final: bf16 datapath, pipelined emission).

Self-contained: builds a Bass/Tile SPMD program for 8 NeuronCores, shards the
batch (conversation) axis across cores, runs via run_bass_kernel_spmd, and
gathers the valid positions on the host.

Math restructuring (same as v1 baseline, verified vs reference):
  - fc layer folded into step-1 LSTM gates:   gates1 = x @ (w_ih @ fc_w).T + (w_ih@fc_b + b)
  - step-1 has h=c=0: f-gate and the w_hh matmul are dead -> skipped
  - step-2: h1 appears both in qstar and via w_hh:  gates2 = h1 @ (w_ih[:, :D] + w_hh).T + r1 @ w_ih[:, D:].T + b
  - softmax normalization deferred to r:  r = (X^T A) * (1/sum_u A),  A = exp(e + mask)
  - sigmoid computed as 0.5*tanh(x/2)+0.5 so the whole main body uses the
    exp_and_others ACT table set (tanh+exp); h and c are carried scaled by 2
    (hs=2h, cs=2c) with compensations folded into host-side weights and the
    free input-scale of downstream activations.

Performance structure (measured ~619us/core vs 1125us baseline):
  - all matmul operands and SBUF intermediates are bf16 (PSUM stays fp32):
    halves DMA and LDWEIGHTS time; rel err 3.6e-4 vs the 2e-2 gate.
  - softmax 1/sum via nc.vector.reciprocal_approx_fast (one custom-DVE op,
    ~5x faster than the iterative InstReciprocal).
  - attention bank chunks use full 128 rows (host data is real beyond the
    conversation length; the additive -30000 exp mask zeroes those rows).
  - log-softmax head: per-conv logits land at psum partition base 32*(j%3)
    (PE quadrant 3 is unusable for outputs); exp+sum run in the main loop
    (exp_and_others table set), sums pack into one [65, 6*T] tile, a single
    Ln covers convs 0..14 and hides under the last (longest) conversation,
    then per 3-conv chunk one selector matmul broadcasts ln-sums and one
    STT forms log-probs.  No DRAM scratch, no barrier.
  - two-stage software-pipelined emission: front(j) = bank DMA + gates1 +
    h1 chain is emitted before back(j-1) = attention/gates2/head, so the
    tensor engine always has independent matmuls in its static program
    order while conv j-1's ACT/DVE chains drain.
  - slots are processed shortest-first so the startup DMA is small and the
    final conversation (longest) hides the log-softmax tail.
  The tensor engine is the bottleneck at ~85% occupancy; its ~450us of
  matmul column streaming (1.08M columns @ 2.4GHz) is the roofline for
  this math.
"""
import os
import sys
sys.path.insert(0, '/opt/trn_rl_repo')

# run_bass_kernel_spmd executes through jax/PJRT on the axon-tunneled
# NeuronCores; a JAX_PLATFORMS=cpu pin would hide them.
if os.environ.get('JAX_PLATFORMS') == 'cpu' and 'jax' not in sys.modules:
    del os.environ['JAX_PLATFORMS']

import numpy as np
import ml_dtypes

BF16 = np.dtype(ml_dtypes.bfloat16)

T_MAX, BATCH, D, C = 512, 128, 256, 7
NCORE = 8
NCONV = BATCH // NCORE          # conversations per core
MASKV = -30000.0                # additive pre-exp mask for invalid bank rows

_BUILD_CACHE = {}


def _build(with_bias1, with_bias2, slot_lens):
    """Build + compile the SPMD Bass program. Returns the Bacc instance."""
    from contextlib import ExitStack
    import concourse.bacc as bacc
    import concourse.bass as bass  # noqa: F401
    from concourse import mybir, tile

    f32 = mybir.dt.float32
    f32r = mybir.dt.float32r
    bf16 = mybir.dt.bfloat16
    AF = mybir.ActivationFunctionType
    ALU = mybir.AluOpType

    nc = bacc.Bacc("TRN2", target_bir_lowering=False, debug=False,
                   num_devices=NCORE)

    def din(name, shape, dt=bf16):
        return nc.dram_tensor(name, shape, dt, kind="ExternalInput").ap()

    xs_d = din("xs", [T_MAX, NCONV, D])
    xp_d = din("xp", [T_MAX, NCONV, D])
    xst_d = din("xst", [NCONV, 2, 128, T_MAX])   # host-pretransposed d-major banks
    xpt_d = din("xpt", [NCONV, 2, 128, T_MAX])
    mask_d = din("mask", [128, NCONV * 4], f32)
    wdefs = {}
    for st in ("s", "p"):
        wdefs[st] = dict(
            we=din(f"we_{st}", [D, 768]),      # (w_ih@fc_w).T, i/g/o rows only
            wh=din(f"wh_{st}", [D, 1024]),     # 0.5*(w_ih[:, :D] + w_hh).T
            wr=din(f"wr_{st}", [D, 1024]),     # w_ih[:, D:].T
            b1=din(f"b1_{st}", [1, 768]),
            b2=din(f"b2_{st}", [1, 1024]),
        )
    ones_d = din("ones_in", [128, 128])
    sel_d = din("sel71", [65, 71], f32)        # ln-sum row -> class-row bcast
    outw_d = din("outw", [4 * D, C])           # adjusted out_w.T (h-cols * 0.5)
    outb_d = din("outb", [128, 1], f32)        # out_b replicated at rows 32i+c
    out_d = nc.dram_tensor("out", [NCONV, C, T_MAX], f32,
                           kind="ExternalOutput").ap()

    UTs = [(int(lv) + 127) // 128 for lv in slot_lens]
    Ls = [min(T_MAX, ((int(lv) + 15) // 16) * 16) for lv in slot_lens]

    with ExitStack() as ctx:
        tc = ctx.enter_context(tile.TileContext(nc))
        const = ctx.enter_context(tc.tile_pool(name="const", bufs=1))
        xpool = ctx.enter_context(tc.tile_pool(name="xpool", bufs=5))
        work = ctx.enter_context(tc.tile_pool(name="work", bufs=2))
        fpool = ctx.enter_context(tc.tile_pool(name="fpool", bufs=1))
        lpool = ctx.enter_context(tc.tile_pool(name="lpool", bufs=1))
        gpsum = ctx.enter_context(tc.tile_pool(name="gpsum", bufs=2, space="PSUM"))
        epsum = ctx.enter_context(tc.tile_pool(name="epsum", bufs=2, space="PSUM"))
        spsum = ctx.enter_context(tc.tile_pool(name="spsum", bufs=1, space="PSUM"))
        rpsum = ctx.enter_context(tc.tile_pool(name="rpsum", bufs=1, space="PSUM"))

        # ---- constants / weights.  we/mask load immediately (first conv
        # needs them); the rest defer to the gpsimd queue after the first
        # two conversations' bank loads are in flight ---------------------
        deferred_dmas = []
        W = {}
        for sti, st in enumerate(("s", "p")):
            d = wdefs[st]
            we_t = const.tile([128, 2, 768], bf16, name=f"we_t{st}")
            nc.sync.dma_start(out=we_t, in_=d["we"].rearrange("(kt p) m -> p kt m", p=128))
            wh_t = const.tile([128, 2, 1024], bf16, name=f"wh_t{st}")
            deferred_dmas.append((wh_t, d["wh"].rearrange("(kt p) m -> p kt m", p=128)))
            wr_t = const.tile([128, 2, 1024], bf16, name=f"wr_t{st}")
            deferred_dmas.append((wr_t, d["wr"].rearrange("(kt p) m -> p kt m", p=128)))
            b1_t = const.tile([1, 768], bf16, name=f"b1_t{st}") if with_bias1 else None
            if with_bias1:
                nc.gpsimd.dma_start(out=b1_t, in_=d["b1"])
            b2_t = const.tile([1, 1024], bf16, name=f"b2_t{st}") if with_bias2 else None
            if with_bias2:
                nc.gpsimd.dma_start(out=b2_t, in_=d["b2"])
            W[sti] = dict(we=we_t, wh=wh_t, wr=wr_t, b1=b1_t, b2=b2_t)
        ones = const.tile([128, 128], bf16)
        deferred_dmas.append((ones, ones_d))
        if with_bias1 or with_bias2:
            onesrow = const.tile([1, T_MAX], bf16)
            nc.gpsimd.dma_start(
                out=onesrow,
                in_=ones_d.rearrange("a b -> (a b)")[0:T_MAX])

        mask_t = const.tile([128, NCONV * 4], f32)
        nc.sync.dma_start(out=mask_t, in_=mask_d)
        outw_t = const.tile([128, 8, C], bf16)
        deferred_dmas.append((outw_t, outw_d.rearrange("(kt p) c -> p kt c", p=128)))
        outb_t = const.tile([128, 1], f32)
        deferred_dmas.append((outb_t, outb_d))
        sel_t = const.tile([65, 71], f32r)
        deferred_dmas.append((sel_t, sel_d.bitcast(f32r)))

        # per-conv exp-sums: conv j -> partition 32*(j%3), col block j//3.
        # junk entries stay at ln(1)=0  (PE output quadrant 3 is unusable,
        # so only partition bases 0/32/64 -> chunks of 3 conversations)
        NCH = (NCONV + 2) // 3
        srows = fpool.tile([65, NCH * T_MAX], f32, name="srows")
        nc.gpsimd.memset(srows, 1.0)
        # packed (logits + out_b), written per conv, read by the tail STT
        lgb = fpool.tile([71, NCH * T_MAX], f32, name="lgb")

        def mm(ps, lhsT, rhs, start, stop):
            nc.tensor.matmul(ps, lhsT, rhs, start=start, stop=stop)

        def attention(j, st, xt, h_t, step, L, UT):
            """A = exp(0.5*e + mask); returns (A tile, Z tile) with Z=1/rowsum."""
            A = work.tile([128, UT, L], bf16, tag="A", bufs=4, name=f"A{j}_{st}_{step}")
            for ut in range(UT):
                pe = epsum.tile([128, T_MAX], f32, tag="pe", name=f"pe{j}_{st}_{step}_{ut}")
                for kd in range(2):
                    mm(pe[:, 0:L], xt[:, kd, ut * 128:(ut + 1) * 128],
                       h_t[:, kd, 0:L], kd == 0, kd == 1)
                col = j * 4 + ut
                nc.scalar.activation(A[:, ut, :], pe[:, 0:L], AF.Exp,
                                     bias=mask_t[:, col:col + 1], scale=0.5)
            psm = spsum.tile([128, T_MAX], f32, tag="psm", name=f"psm{j}_{st}_{step}")
            for ut in range(UT):
                mm(psm[:, 0:L], ones, A[:, ut, :], ut == 0, ut == UT - 1)
            Z = work.tile([128, L], f32, tag="Z", bufs=3, name=f"Z{j}_{st}_{step}")
            nc.vector.reciprocal_approx_fast(Z, psm[:, 0:L])
            return A, Z

        def r_matmul(j, st, xn, A, Z, out_tile, out_zoff, relu, L, UT):
            """out[:, out_zoff+dt, :] = (X^T A) * Z  (optionally relu'd)."""
            for dt in range(2):
                pr = rpsum.tile([128, T_MAX], f32, tag="pr", name=f"pr{j}_{st}_{out_zoff}_{dt}")
                for ut in range(UT):
                    mm(pr[:, 0:L], xn[:, ut, dt * 128:(dt + 1) * 128],
                       A[:, ut, :], ut == 0, ut == UT - 1)
                if relu:
                    tmpr = work.tile([128, L], f32, tag="tmpr", name=f"tmpr{j}_{st}_{dt}")
                    nc.vector.tensor_mul(tmpr, pr[:, 0:L], Z)
                    nc.vector.tensor_scalar_max(out_tile[:, out_zoff + dt, :],
                                                tmpr, 0.0)
                else:
                    nc.vector.tensor_mul(out_tile[:, out_zoff + dt, :],
                                         pr[:, 0:L], Z)

        lns = fpool.tile([65, NCH * T_MAX], f32r, name="lns")

        def _tail_chunks(ccs):
            """Ln over the given chunk col-range, then log-prob + DMA out."""
            c0, c1 = ccs[0], ccs[-1] + 1
            nc.scalar.activation(lns[:, c0 * T_MAX:c1 * T_MAX],
                                 srows[:, c0 * T_MAX:c1 * T_MAX], AF.Ln)
            for cc in ccs:
                Lc = max(Ls[cc * 3:min(cc * 3 + 3, NCONV)])
                lnsb = epsum.tile([71, T_MAX], f32, tag="pe", name=f"lnsb{cc}")
                mm(lnsb[:, 0:Lc], sel_t, lns[:, cc * T_MAX:cc * T_MAX + Lc],
                   True, True)
                lp = lpool.tile([71, T_MAX], f32, tag="lp", bufs=2, name=f"lp{cc}")
                nc.vector.scalar_tensor_tensor(
                    lp[:, 0:Lc], lgb[:, cc * T_MAX:cc * T_MAX + Lc], 0.0,
                    lnsb[:, 0:Lc], ALU.add, ALU.subtract)
                for i in range(min(3, NCONV - cc * 3)):
                    jx = cc * 3 + i
                    nc.sync.dma_start(out=out_d[jx, :, 0:Ls[jx]],
                                      in_=lp[32 * i:32 * i + C, 0:Ls[jx]])

        # ---- main loop: 2-stage software pipeline ----------------------
        # front(j) = bank DMA + gates1 + h1 chain; back(j) = attention1,
        # gates2, attention2, logits head.  Emitting front(j) before
        # back(j-1) gives the PE independent matmuls, in program order,
        # to chew on while conv j-1's ACT/DVE chains drain.
        state = {}

        def front(j):
            L = Ls[j]
            UT = UTs[j]
            UC = UT * 128
            XT, XN = {}, {}
            for st in (0, 1):
                src_ = xs_d if st == 0 else xp_d
                srct = xst_d if st == 0 else xpt_d
                xn = xpool.tile([128, 4, D], bf16, tag="xn", name=f"xn{j}_{st}")
                nc.sync.dma_start(
                    out=xn[:, 0:UT, :],
                    in_=src_[:, j, :].rearrange("(ut p) d -> p ut d", p=128)[:, 0:UT, :])
                xt = xpool.tile([128, 2, T_MAX], bf16, tag="xt", name=f"xt{j}_{st}")
                nc.sync.dma_start(
                    out=xt[:, :, 0:UC],
                    in_=srct[j].rearrange("kd p c -> p kd c")[:, :, 0:UC])
                XT[st], XN[st] = xt, xn
            g1_, cs1_, hs1_ = {}, {}, {}
            for st in (0, 1):
                w = W[st]
                xt = XT[st]
                g1 = {}
                for pi, nm in enumerate(("i", "g", "o")):
                    ps = gpsum.tile([128, 2, T_MAX], f32, tag="pg", name=f"pg1{j}_{st}_{pi}")
                    for z in range(2):
                        m = pi * 2 + z
                        for kd in range(2):
                            mm(ps[:, z, 0:L], w["we"][:, kd, m * 128:(m + 1) * 128],
                               xt[:, kd, 0:L], kd == 0, (kd == 1) and not with_bias1)
                        if with_bias1:
                            mm(ps[:, z, 0:L], w["b1"][:, m * 128:(m + 1) * 128],
                               onesrow[0:1, 0:L], False, True)
                    tt = work.tile([128, 2, L], bf16, tag="gact", bufs=10,
                                   name=f"t1{nm}{j}_{st}")
                    nc.scalar.activation(tt[:, :, :], ps[:, :, 0:L], AF.Tanh,
                                         scale=1.0 if nm == "g" else 0.5)
                    g1[nm] = tt
                g1_[st] = g1
            for st in (0, 1):
                g1 = g1_[st]
                cs1 = work.tile([128, 2, L], bf16, tag="cs", bufs=6, name=f"cs1{j}_{st}")
                nc.vector.scalar_tensor_tensor(cs1[:, :, :], g1["i"][:, :, :],
                                               1.0, g1["g"][:, :, :],
                                               ALU.add, ALU.mult)
                th1 = work.tile([128, 2, L], bf16, tag="tmp", bufs=8, name=f"th1{j}_{st}")
                nc.scalar.activation(th1[:, :, :], cs1[:, :, :], AF.Tanh, scale=0.5)
                hs1 = work.tile([128, 2, L], bf16, tag="hs", bufs=7, name=f"hs1{j}_{st}")
                nc.vector.scalar_tensor_tensor(hs1[:, :, :], g1["o"][:, :, :],
                                               1.0, th1[:, :, :],
                                               ALU.add, ALU.mult)
                cs1_[st], hs1_[st] = cs1, hs1
            state[j] = (XT, XN, cs1_, hs1_)

        def back(j):
            XT, XN, cs1_, hs1_ = state.pop(j)
            L = Ls[j]
            UT = UTs[j]
            A1_, Z1_, r1_ = {}, {}, {}
            g2_, hs2_, ft_ = {}, {}, {}
            for st in (0, 1):
                A1, Z1 = attention(j, st, XT[st], hs1_[st], 1, L, UT)
                A1_[st], Z1_[st] = A1, Z1
            for st in (0, 1):
                r1 = work.tile([128, 2, L], bf16, tag="r1", bufs=3, name=f"r1{j}_{st}")
                r_matmul(j, st, XN[st], A1_[st], Z1_[st], r1, 0, False, L, UT)
                r1_[st] = r1
            for st in (0, 1):
                w = W[st]
                g2 = {}
                for pi, nm in enumerate(("i", "f", "g", "o")):
                    ps = gpsum.tile([128, 2, T_MAX], f32, tag="pg", name=f"pg2{j}_{st}_{pi}")
                    for z in range(2):
                        m = pi * 2 + z
                        for kd in range(2):
                            mm(ps[:, z, 0:L], w["wh"][:, kd, m * 128:(m + 1) * 128],
                               hs1_[st][:, kd, :], kd == 0, False)
                        for kd in range(2):
                            mm(ps[:, z, 0:L], w["wr"][:, kd, m * 128:(m + 1) * 128],
                               r1_[st][:, kd, :], False, (kd == 1) and not with_bias2)
                        if with_bias2:
                            mm(ps[:, z, 0:L], w["b2"][:, m * 128:(m + 1) * 128],
                               onesrow[0:1, 0:L], False, True)
                    tt = work.tile([128, 2, L], bf16, tag="gact", bufs=10,
                                   name=f"t2{nm}{j}_{st}")
                    nc.scalar.activation(tt[:, :, :], ps[:, :, 0:L], AF.Tanh,
                                         scale=1.0 if nm == "g" else 0.5)
                    g2[nm] = tt
                g2_[st] = g2
            for st in (0, 1):
                g2, cs1 = g2_[st], cs1_[st]
                t1 = work.tile([128, 2, L], bf16, tag="tmp", bufs=8, name=f"t1_{j}_{st}")
                nc.vector.scalar_tensor_tensor(t1[:, :, :], g2["f"][:, :, :], 1.0,
                                               cs1[:, :, :], ALU.add, ALU.mult)
                t2 = work.tile([128, 2, L], bf16, tag="tmp", bufs=8, name=f"t2_{j}_{st}")
                nc.vector.scalar_tensor_tensor(t2[:, :, :], g2["i"][:, :, :], 1.0,
                                               g2["g"][:, :, :], ALU.add, ALU.mult)
                cs2 = work.tile([128, 2, L], bf16, tag="cs", bufs=6, name=f"cs2{j}_{st}")
                nc.vector.scalar_tensor_tensor(cs2[:, :, :], t1[:, :, :], 0.5,
                                               t2[:, :, :], ALU.mult, ALU.add)
                th2 = work.tile([128, 2, L], bf16, tag="tmp", bufs=8, name=f"th2{j}_{st}")
                nc.scalar.activation(th2[:, :, :], cs2[:, :, :], AF.Tanh, scale=0.5)
                hs2 = work.tile([128, 2, L], bf16, tag="hs", bufs=7, name=f"hs2{j}_{st}")
                nc.vector.scalar_tensor_tensor(hs2[:, :, :], g2["o"][:, :, :],
                                               1.0, th2[:, :, :], ALU.add, ALU.mult)
                hs2_[st] = hs2
            for st in (0, 1):
                A2, Z2 = attention(j, st, XT[st], hs2_[st], 2, L, UT)
                ft = fpool.tile([128, 4, L], bf16, tag=f"feat{st}", bufs=2,
                                name=f"feat{j}_{st}")
                for zz in range(2):
                    nc.vector.tensor_scalar_max(ft[:, zz, :],
                                                hs2_[st][:, zz, :], 0.0)
                r_matmul(j, st, XN[st], A2, Z2, ft, 2, True, L, UT)
                ft_[st] = ft

            # ---- logits + exp-sum for conversation j (exp table set) ----
            pb = 32 * (j % 3)
            cb = (j // 3) * T_MAX
            pl = gpsum.tile([128, T_MAX], f32, tag="pg", name=f"pl{j}")
            for kt in range(8):
                rhs = ft_[kt // 4][:, kt % 4, :]
                mm(pl[pb:pb + C, 0:L], outw_t[:, kt, :], rhs, kt == 0, kt == 7)
            nc.scalar.activation(lgb[pb:pb + C, cb:cb + L], pl[pb:pb + C, 0:L],
                                 AF.Identity, bias=outb_t[pb:pb + C, 0:1])
            elg = work.tile([71, T_MAX], bf16, tag="elg", bufs=2, name=f"elg{j}")
            nc.scalar.activation(elg[pb:pb + C, 0:L], lgb[pb:pb + C, cb:cb + L],
                                 AF.Exp)
            s1 = spsum.tile([128, T_MAX], f32, tag="psm", name=f"s1_{j}")
            mm(s1[pb:pb + 1, 0:L], ones[pb:pb + C, 0:1], elg[pb:pb + C, 0:L],
               True, True)
            nc.scalar.activation(srows[pb:pb + 1, cb:cb + L], s1[pb:pb + 1, 0:L],
                                 AF.Copy)

        for j in range(NCONV):
            front(j)
            if j == 1:
                for dst, srcap in deferred_dmas:
                    nc.gpsimd.dma_start(out=dst, in_=srcap)
            if j > 0:
                back(j - 1)
        # chunks 0..NCH-2 cover convs 0..14: emitted before back(15) so
        # their log-softmax tail hides under the last conversation's compute
        _tail_chunks(list(range(NCH - 1)))
        back(NCONV - 1)
        _tail_chunks([NCH - 1])



    nc.compile()
    return nc


def _host_prep(inputs):
    """Fold weights, pick the conversation->core assignment, build per-core arrays."""
    x_s = np.asarray(inputs["input"], dtype=np.float32)
    x_p = np.asarray(inputs["speakers"], dtype=np.float32)
    lengths = np.asarray(inputs["utterance_lengths"]).astype(np.int64)
    fc_w = np.asarray(inputs["fc_w"], dtype=np.float32)
    fc_b = np.asarray(inputs["fc_b"], dtype=np.float32)
    out_w = np.asarray(inputs["out_w"], dtype=np.float32)
    out_b = np.asarray(inputs["out_b"], dtype=np.float32)

    per_stream = {}
    any_b1 = False
    any_b2 = False
    for st in ("s", "p"):
        w_ih = np.asarray(inputs[f"w_ih_{st}"], dtype=np.float32)
        w_hh = np.asarray(inputs[f"w_hh_{st}"], dtype=np.float32)
        b_ih = np.asarray(inputs[f"b_ih_{st}"], dtype=np.float32)
        b_hh = np.asarray(inputs[f"b_hh_{st}"], dtype=np.float32)
        W_eff = w_ih @ fc_w                          # [1024, 256]
        bias1 = w_ih @ fc_b + b_ih + b_hh            # [1024]
        sel = np.r_[0:D, 2 * D:4 * D]                # i, g, o rows
        We_sel = np.ascontiguousarray(W_eff[sel].T).astype(BF16)  # [256, 768]
        b1_sel = np.ascontiguousarray(bias1[sel])[None, :].astype(BF16)
        Wh = np.ascontiguousarray((0.5 * (w_ih[:, :D] + w_hh)).T).astype(BF16)
        Wr = np.ascontiguousarray(w_ih[:, D:].T).astype(BF16)     # [256, 1024]
        b2 = np.ascontiguousarray(b_ih + b_hh)[None, :].astype(BF16)
        per_stream[st] = (We_sel, Wh, Wr, b1_sel, b2)
        any_b1 |= bool(np.any(np.asarray(b1_sel, np.float32) != 0.0))
        any_b2 |= bool(np.any(np.asarray(b2, np.float32) != 0.0))

    # out_w columns for the h-halves get the 0.5 compensation (h is stored as 2h)
    ow = out_w.copy()
    ow[:, 0:D] *= 0.5
    ow[:, 2 * D:3 * D] *= 0.5
    outw = np.ascontiguousarray(ow.T).astype(BF16)   # [1024, 7]
    outb = np.zeros((128, 1), dtype=np.float32)      # replicated at rows 32i+c
    for i in range(3):
        outb[32 * i:32 * i + C, 0] = out_b

    # selector: ln-sum at partition 32i -> class rows 32i..32i+6
    sel71 = np.zeros((65, 71), dtype=np.float32)
    for i in range(3):
        sel71[32 * i, 32 * i:32 * i + C] = 1.0

    # conversation -> (core, slot): sort by length desc, round-robin over cores
    order = np.argsort(-lengths, kind="stable")
    assign = {}   # conv -> (core, slot); slot 0 = shortest, last = longest
    for rank, conv in enumerate(order):
        assign[int(conv)] = (rank % NCORE, NCONV - 1 - rank // NCORE)

    order_lens = lengths[order]
    slot_lens = tuple(int(order_lens[8 * (NCONV - 1 - k)])
                      for k in range(NCONV))

    x_s16 = x_s.astype(BF16)
    x_p16 = x_p.astype(BF16)

    in_maps = []
    core_convs = []
    for core in range(NCORE):
        ids = [None] * NCONV
        for conv, (c, s) in assign.items():
            if c == core:
                ids[s] = conv
        core_convs.append(ids)
        mask = np.zeros((128, NCONV * 4), dtype=np.float32)
        for s, conv in enumerate(ids):
            Lc = int(lengths[conv])
            u = np.arange(T_MAX)
            m = np.where(u < Lc, 0.0, MASKV).astype(np.float32)
            mask[:, s * 4:(s + 1) * 4] = m.reshape(4, 128).T
        im = {
            "xs": np.ascontiguousarray(x_s16[:, ids, :]),
            "xp": np.ascontiguousarray(x_p16[:, ids, :]),
            "xst": np.ascontiguousarray(
                x_s16[:, ids, :].transpose(1, 2, 0).reshape(NCONV, 2, 128, T_MAX)),
            "xpt": np.ascontiguousarray(
                x_p16[:, ids, :].transpose(1, 2, 0).reshape(NCONV, 2, 128, T_MAX)),
            "mask": mask,
            "ones_in": np.ones((128, 128), dtype=BF16),
            "sel71": sel71,
            "outw": outw,
            "outb": outb,
        }
        for st in ("s", "p"):
            We_sel, Wh, Wr, b1_sel, b2 = per_stream[st]
            im[f"we_{st}"] = We_sel
            im[f"wh_{st}"] = Wh
            im[f"wr_{st}"] = Wr
            im[f"b1_{st}"] = b1_sel
            im[f"b2_{st}"] = b2
        in_maps.append(im)
    return in_maps, core_convs, lengths, any_b1, any_b2, slot_lens


def _gather(results, core_convs, lengths):
    """results: list (per core) of {'out': [NCONV, C, T_MAX]} -> [sum(len), C]."""
    where = {}
    for core, ids in enumerate(core_convs):
        for slot, conv in enumerate(ids):
            where[conv] = (core, slot)
    chunks = []
    for b in range(BATCH):
        core, slot = where[b]
        L = int(lengths[b])
        chunks.append(np.ascontiguousarray(results[core]["out"][slot, :, :L].T))
    return np.concatenate(chunks, axis=0).astype(np.float32)


def _get_nc(any_b1, any_b2, slot_lens):
    key = (any_b1, any_b2, slot_lens)
    if key not in _BUILD_CACHE:
        _BUILD_CACHE[key] = _build(any_b1, any_b2, slot_lens)
    return _BUILD_CACHE[key]


def kernel(**inputs):
    from concourse import bass_utils
    in_maps, core_convs, lengths, any_b1, any_b2, slot_lens = _host_prep(inputs)
    nc = _get_nc(any_b1, any_b2, slot_lens)
    res = bass_utils.run_bass_kernel_spmd(nc, in_maps, core_ids=list(range(NCORE)))
    return _gather(res.results, core_convs, lengths)


# revision 25
# speedup vs baseline: 1.0387x; 1.0387x over previous
"""DCRNCognition Trainium2 kernel (final: bf16 datapath, pipelined emission).

Self-contained: builds a Bass/Tile SPMD program for 8 NeuronCores, shards the
batch (conversation) axis across cores, runs via run_bass_kernel_spmd, and
gathers the valid positions on the host.

Math restructuring (same as v1 baseline, verified vs reference):
  - fc layer folded into step-1 LSTM gates:   gates1 = x @ (w_ih @ fc_w).T + (w_ih@fc_b + b)
  - step-1 has h=c=0: f-gate and the w_hh matmul are dead -> skipped
  - step-2: h1 appears both in qstar and via w_hh:  gates2 = h1 @ (w_ih[:, :D] + w_hh).T + r1 @ w_ih[:, D:].T + b
  - softmax normalization deferred to r:  r = (X^T A) * (1/sum_u A),  A = exp(e + mask)
  - sigmoid computed as 0.5*tanh(x/2)+0.5 so the whole main body uses the
    exp_and_others ACT table set (tanh+exp); h and c are carried scaled by 2
    (hs=2h, cs=2c) with compensations folded into host-side weights and the
    free input-scale of downstream activations.

Performance structure (measured ~619us/core vs 1125us baseline):
  - all matmul operands and SBUF intermediates are bf16 (PSUM stays fp32):
    halves DMA and LDWEIGHTS time; rel err 3.6e-4 vs the 2e-2 gate.
  - softmax 1/sum via nc.vector.reciprocal_approx_fast (one custom-DVE op,
    ~5x faster than the iterative InstReciprocal).
  - attention bank chunks use full 128 rows (host data is real beyond the
    conversation length; the additive -30000 exp mask zeroes those rows).
  - log-softmax head: per-conv logits land at psum partition base 32*(j%3)
    (PE quadrant 3 is unusable for outputs); exp+sum run in the main loop
    (exp_and_others table set), sums pack into one [65, 6*T] tile, a single
    Ln covers convs 0..14 and hides under the last (longest) conversation,
    then per 3-conv chunk one selector matmul broadcasts ln-sums and one
    STT forms log-probs.  No DRAM scratch, no barrier.
  - two-stage software-pipelined emission: front(j) = bank DMA + gates1 +
    h1 chain is emitted before back(j-1) = attention/gates2/head, so the
    tensor engine always has independent matmuls in its static program
    order while conv j-1's ACT/DVE chains drain.
  - slots are processed shortest-first so the startup DMA is small and the
    final conversation (longest) hides the log-softmax tail.
  The tensor engine is the bottleneck at ~85% occupancy; its ~450us of
  matmul column streaming (1.08M columns @ 2.4GHz) is the roofline for
  this math.
"""
import os
import sys
sys.path.insert(0, '/opt/trn_rl_repo')

# run_bass_kernel_spmd executes through jax/PJRT on the axon-tunneled
# NeuronCores; a JAX_PLATFORMS=cpu pin would hide them.
if os.environ.get('JAX_PLATFORMS') == 'cpu' and 'jax' not in sys.modules:
    del os.environ['JAX_PLATFORMS']

import numpy as np
import ml_dtypes

BF16 = np.dtype(ml_dtypes.bfloat16)

T_MAX, BATCH, D, C = 512, 128, 256, 7
NCORE = 8
NCONV = BATCH // NCORE          # conversations per core
MASKV = -30000.0                # additive pre-exp mask for invalid bank rows

_BUILD_CACHE = {}


def _build(with_bias1, with_bias2, slot_lens):
    """Build + compile the SPMD Bass program. Returns the Bacc instance."""
    from contextlib import ExitStack
    import concourse.bacc as bacc
    import concourse.bass as bass  # noqa: F401
    from concourse import mybir, tile

    f32 = mybir.dt.float32
    f32r = mybir.dt.float32r
    bf16 = mybir.dt.bfloat16
    AF = mybir.ActivationFunctionType
    ALU = mybir.AluOpType

    nc = bacc.Bacc("TRN2", target_bir_lowering=False, debug=False,
                   num_devices=NCORE)

    def din(name, shape, dt=bf16):
        return nc.dram_tensor(name, shape, dt, kind="ExternalInput").ap()

    xs_d = din("xs", [T_MAX, NCONV, D])
    xp_d = din("xp", [T_MAX, NCONV, D])
    xst_d = din("xst", [NCONV, 2, 128, T_MAX])   # host-pretransposed d-major banks
    xpt_d = din("xpt", [NCONV, 2, 128, T_MAX])
    mask_d = din("mask", [128, NCONV * 4], f32)
    wdefs = {}
    for st in ("s", "p"):
        wdefs[st] = dict(
            we=din(f"we_{st}", [D, 768]),      # (w_ih@fc_w).T, i/g/o rows only
            wh=din(f"wh_{st}", [D, 1024]),     # 0.5*(w_ih[:, :D] + w_hh).T
            wr=din(f"wr_{st}", [D, 1024]),     # w_ih[:, D:].T
            b1=din(f"b1_{st}", [1, 768]),
            b2=din(f"b2_{st}", [1, 1024]),
        )
    ones_d = din("ones_in", [128, 128])
    sel_d = din("sel71", [65, 71], f32)        # ln-sum row -> class-row bcast
    outw_d = din("outw", [4 * D, C])           # adjusted out_w.T (h-cols * 0.5)
    outb_d = din("outb", [128, 1], f32)        # out_b replicated at rows 32i+c
    out_d = nc.dram_tensor("out", [NCONV, C, T_MAX], f32,
                           kind="ExternalOutput").ap()

    UTs = [(int(lv) + 127) // 128 for lv in slot_lens]
    Ls = [min(T_MAX, ((int(lv) + 15) // 16) * 16) for lv in slot_lens]

    with ExitStack() as ctx:
        tc = ctx.enter_context(tile.TileContext(nc))
        const = ctx.enter_context(tc.tile_pool(name="const", bufs=1))
        xpool = ctx.enter_context(tc.tile_pool(name="xpool", bufs=5))
        work = ctx.enter_context(tc.tile_pool(name="work", bufs=2))
        fpool = ctx.enter_context(tc.tile_pool(name="fpool", bufs=1))
        lpool = ctx.enter_context(tc.tile_pool(name="lpool", bufs=1))
        gpsum = ctx.enter_context(tc.tile_pool(name="gpsum", bufs=4, space="PSUM"))
        epsum = ctx.enter_context(tc.tile_pool(name="epsum", bufs=2, space="PSUM"))
        spsum = ctx.enter_context(tc.tile_pool(name="spsum", bufs=1, space="PSUM"))
        rpsum = ctx.enter_context(tc.tile_pool(name="rpsum", bufs=1, space="PSUM"))

        # ---- constants / weights.  we/mask load immediately (first conv
        # needs them); the rest defer to the gpsimd queue after the first
        # two conversations' bank loads are in flight ---------------------
        deferred_dmas = []
        W = {}
        for sti, st in enumerate(("s", "p")):
            d = wdefs[st]
            we_t = const.tile([128, 2, 768], bf16, name=f"we_t{st}")
            nc.sync.dma_start(out=we_t, in_=d["we"].rearrange("(kt p) m -> p kt m", p=128))
            wh_t = const.tile([128, 2, 1024], bf16, name=f"wh_t{st}")
            deferred_dmas.append((wh_t, d["wh"].rearrange("(kt p) m -> p kt m", p=128)))
            wr_t = const.tile([128, 2, 1024], bf16, name=f"wr_t{st}")
            deferred_dmas.append((wr_t, d["wr"].rearrange("(kt p) m -> p kt m", p=128)))
            b1_t = const.tile([1, 768], bf16, name=f"b1_t{st}") if with_bias1 else None
            if with_bias1:
                nc.gpsimd.dma_start(out=b1_t, in_=d["b1"])
            b2_t = const.tile([1, 1024], bf16, name=f"b2_t{st}") if with_bias2 else None
            if with_bias2:
                nc.gpsimd.dma_start(out=b2_t, in_=d["b2"])
            W[sti] = dict(we=we_t, wh=wh_t, wr=wr_t, b1=b1_t, b2=b2_t)
        ones = const.tile([128, 128], bf16)
        deferred_dmas.append((ones, ones_d))
        if with_bias1 or with_bias2:
            onesrow = const.tile([1, T_MAX], bf16)
            nc.gpsimd.dma_start(
                out=onesrow,
                in_=ones_d.rearrange("a b -> (a b)")[0:T_MAX])

        mask_t = const.tile([128, NCONV * 4], f32)
        nc.sync.dma_start(out=mask_t, in_=mask_d)
        outw_t = const.tile([128, 8, C], bf16)
        deferred_dmas.append((outw_t, outw_d.rearrange("(kt p) c -> p kt c", p=128)))
        outb_t = const.tile([128, 1], f32)
        deferred_dmas.append((outb_t, outb_d))
        sel_t = const.tile([65, 71], f32r)
        deferred_dmas.append((sel_t, sel_d.bitcast(f32r)))

        # per-conv exp-sums: conv j -> partition 32*(j%3), col block j//3.
        # junk entries stay at ln(1)=0  (PE output quadrant 3 is unusable,
        # so only partition bases 0/32/64 -> chunks of 3 conversations)
        NCH = (NCONV + 2) // 3
        srows = fpool.tile([65, NCH * T_MAX], f32, name="srows")
        nc.gpsimd.memset(srows, 1.0)
        # packed (logits + out_b), written per conv, read by the tail STT
        lgb = fpool.tile([71, NCH * T_MAX], f32, name="lgb")

        def mm(ps, lhsT, rhs, start, stop):
            nc.tensor.matmul(ps, lhsT, rhs, start=start, stop=stop)

        def attention(j, st, xt, h_t, step, L, UT):
            """A = exp(0.5*e + mask); returns (A tile, Z tile) with Z=1/rowsum."""
            A = work.tile([128, UT, L], bf16, tag="A", bufs=4, name=f"A{j}_{st}_{step}")
            for ut in range(UT):
                pe = epsum.tile([128, T_MAX], f32, tag="pe", name=f"pe{j}_{st}_{step}_{ut}")
                for kd in range(2):
                    mm(pe[:, 0:L], xt[:, kd, ut * 128:(ut + 1) * 128],
                       h_t[:, kd, 0:L], kd == 0, kd == 1)
                col = j * 4 + ut
                nc.scalar.activation(A[:, ut, :], pe[:, 0:L], AF.Exp,
                                     bias=mask_t[:, col:col + 1], scale=0.5)
            psm = spsum.tile([128, T_MAX], f32, tag="psm", name=f"psm{j}_{st}_{step}")
            for ut in range(UT):
                mm(psm[:, 0:L], ones, A[:, ut, :], ut == 0, ut == UT - 1)
            Z = work.tile([128, L], f32, tag="Z", bufs=3, name=f"Z{j}_{st}_{step}")
            nc.vector.reciprocal_approx_fast(Z, psm[:, 0:L])
            return A, Z

        def r_matmul(j, st, xn, A, Z, out_tile, out_zoff, relu, L, UT):
            """out[:, out_zoff+dt, :] = (X^T A) * Z  (optionally relu'd)."""
            for dt in range(2):
                pr = rpsum.tile([128, T_MAX], f32, tag="pr", name=f"pr{j}_{st}_{out_zoff}_{dt}")
                for ut in range(UT):
                    mm(pr[:, 0:L], xn[:, ut, dt * 128:(dt + 1) * 128],
                       A[:, ut, :], ut == 0, ut == UT - 1)
                if relu:
                    tmpr = work.tile([128, L], f32, tag="tmpr", name=f"tmpr{j}_{st}_{dt}")
                    nc.vector.tensor_mul(tmpr, pr[:, 0:L], Z)
                    nc.vector.tensor_scalar_max(out_tile[:, out_zoff + dt, :],
                                                tmpr, 0.0)
                else:
                    nc.vector.tensor_mul(out_tile[:, out_zoff + dt, :],
                                         pr[:, 0:L], Z)

        lns = fpool.tile([65, NCH * T_MAX], f32r, name="lns")

        def _tail_chunks(ccs):
            """Ln over the given chunk col-range, then log-prob + DMA out."""
            c0, c1 = ccs[0], ccs[-1] + 1
            nc.scalar.activation(lns[:, c0 * T_MAX:c1 * T_MAX],
                                 srows[:, c0 * T_MAX:c1 * T_MAX], AF.Ln)
            for cc in ccs:
                Lc = max(Ls[cc * 3:min(cc * 3 + 3, NCONV)])
                lnsb = epsum.tile([71, T_MAX], f32, tag="pe", name=f"lnsb{cc}")
                mm(lnsb[:, 0:Lc], sel_t, lns[:, cc * T_MAX:cc * T_MAX + Lc],
                   True, True)
                lp = lpool.tile([71, T_MAX], f32, tag="lp", bufs=2, name=f"lp{cc}")
                nc.vector.scalar_tensor_tensor(
                    lp[:, 0:Lc], lgb[:, cc * T_MAX:cc * T_MAX + Lc], 0.0,
                    lnsb[:, 0:Lc], ALU.add, ALU.subtract)
                for i in range(min(3, NCONV - cc * 3)):
                    jx = cc * 3 + i
                    nc.sync.dma_start(out=out_d[jx, :, 0:Ls[jx]],
                                      in_=lp[32 * i:32 * i + C, 0:Ls[jx]])

        # ---- main loop: 2-stage software pipeline ----------------------
        # front(j) = bank DMA + gates1 + h1 chain; back(j) = attention1,
        # gates2, attention2, logits head.  Emitting front(j) before
        # back(j-1) gives the PE independent matmuls, in program order,
        # to chew on while conv j-1's ACT/DVE chains drain.
        state = {}

        def front(j):
            L = Ls[j]
            UT = UTs[j]
            UC = UT * 128
            XT, XN = {}, {}
            for st in (0, 1):
                src_ = xs_d if st == 0 else xp_d
                srct = xst_d if st == 0 else xpt_d
                xn = xpool.tile([128, 4, D], bf16, tag="xn", name=f"xn{j}_{st}")
                nc.sync.dma_start(
                    out=xn[:, 0:UT, :],
                    in_=src_[:, j, :].rearrange("(ut p) d -> p ut d", p=128)[:, 0:UT, :])
                xt = xpool.tile([128, 2, T_MAX], bf16, tag="xt", name=f"xt{j}_{st}")
                nc.sync.dma_start(
                    out=xt[:, :, 0:UC],
                    in_=srct[j].rearrange("kd p c -> p kd c")[:, :, 0:UC])
                XT[st], XN[st] = xt, xn
            g1_, cs1_, hs1_ = {}, {}, {}
            for st in (0, 1):
                w = W[st]
                xt = XT[st]
                g1 = {}
                for pi, nm in enumerate(("i", "g", "o")):
                    tt = work.tile([128, 2, L], bf16, tag="gact", bufs=10,
                                   name=f"t1{nm}{j}_{st}")
                    for z in range(2):
                        m = pi * 2 + z
                        ps = gpsum.tile([128, T_MAX], f32, tag="pg",
                                        name=f"pg1{j}_{st}_{pi}_{z}")
                        for kd in range(2):
                            mm(ps[:, 0:L], w["we"][:, kd, m * 128:(m + 1) * 128],
                               xt[:, kd, 0:L], kd == 0, (kd == 1) and not with_bias1)
                        if with_bias1:
                            mm(ps[:, 0:L], w["b1"][:, m * 128:(m + 1) * 128],
                               onesrow[0:1, 0:L], False, True)
                        nc.scalar.activation(tt[:, z, :], ps[:, 0:L], AF.Tanh,
                                             scale=1.0 if nm == "g" else 0.5)
                    g1[nm] = tt
                g1_[st] = g1
            for st in (0, 1):
                g1 = g1_[st]
                cs1 = work.tile([128, 2, L], bf16, tag="cs", bufs=6, name=f"cs1{j}_{st}")
                nc.vector.scalar_tensor_tensor(cs1[:, :, :], g1["i"][:, :, :],
                                               1.0, g1["g"][:, :, :],
                                               ALU.add, ALU.mult)
                th1 = work.tile([128, 2, L], bf16, tag="tmp", bufs=8, name=f"th1{j}_{st}")
                nc.scalar.activation(th1[:, :, :], cs1[:, :, :], AF.Tanh, scale=0.5)
                hs1 = work.tile([128, 2, L], bf16, tag="hs", bufs=7, name=f"hs1{j}_{st}")
                nc.vector.scalar_tensor_tensor(hs1[:, :, :], g1["o"][:, :, :],
                                               1.0, th1[:, :, :],
                                               ALU.add, ALU.mult)
                cs1_[st], hs1_[st] = cs1, hs1
            state[j] = (XT, XN, cs1_, hs1_)

        def back(j):
            XT, XN, cs1_, hs1_ = state.pop(j)
            L = Ls[j]
            UT = UTs[j]
            A1_, Z1_, r1_ = {}, {}, {}
            g2_, hs2_, ft_ = {}, {}, {}
            for st in (0, 1):
                A1, Z1 = attention(j, st, XT[st], hs1_[st], 1, L, UT)
                A1_[st], Z1_[st] = A1, Z1
            for st in (0, 1):
                r1 = work.tile([128, 2, L], bf16, tag="r1", bufs=3, name=f"r1{j}_{st}")
                r_matmul(j, st, XN[st], A1_[st], Z1_[st], r1, 0, False, L, UT)
                r1_[st] = r1
            for st in (0, 1):
                w = W[st]
                g2 = {}
                for pi, nm in enumerate(("i", "f", "g", "o")):
                    tt = work.tile([128, 2, L], bf16, tag="gact", bufs=10,
                                   name=f"t2{nm}{j}_{st}")
                    for z in range(2):
                        m = pi * 2 + z
                        ps = gpsum.tile([128, T_MAX], f32, tag="pg",
                                        name=f"pg2{j}_{st}_{pi}_{z}")
                        for kd in range(2):
                            mm(ps[:, 0:L], w["wh"][:, kd, m * 128:(m + 1) * 128],
                               hs1_[st][:, kd, :], kd == 0, False)
                        for kd in range(2):
                            mm(ps[:, 0:L], w["wr"][:, kd, m * 128:(m + 1) * 128],
                               r1_[st][:, kd, :], False, (kd == 1) and not with_bias2)
                        if with_bias2:
                            mm(ps[:, 0:L], w["b2"][:, m * 128:(m + 1) * 128],
                               onesrow[0:1, 0:L], False, True)
                        nc.scalar.activation(tt[:, z, :], ps[:, 0:L], AF.Tanh,
                                             scale=1.0 if nm == "g" else 0.5)
                    g2[nm] = tt
                g2_[st] = g2
            for st in (0, 1):
                g2, cs1 = g2_[st], cs1_[st]
                t1 = work.tile([128, 2, L], bf16, tag="tmp", bufs=8, name=f"t1_{j}_{st}")
                nc.vector.scalar_tensor_tensor(t1[:, :, :], g2["f"][:, :, :], 1.0,
                                               cs1[:, :, :], ALU.add, ALU.mult)
                t2 = work.tile([128, 2, L], bf16, tag="tmp", bufs=8, name=f"t2_{j}_{st}")
                nc.vector.scalar_tensor_tensor(t2[:, :, :], g2["i"][:, :, :], 1.0,
                                               g2["g"][:, :, :], ALU.add, ALU.mult)
                cs2 = work.tile([128, 2, L], bf16, tag="cs", bufs=6, name=f"cs2{j}_{st}")
                nc.vector.scalar_tensor_tensor(cs2[:, :, :], t1[:, :, :], 0.5,
                                               t2[:, :, :], ALU.mult, ALU.add)
                th2 = work.tile([128, 2, L], bf16, tag="tmp", bufs=8, name=f"th2{j}_{st}")
                nc.scalar.activation(th2[:, :, :], cs2[:, :, :], AF.Tanh, scale=0.5)
                hs2 = work.tile([128, 2, L], bf16, tag="hs", bufs=7, name=f"hs2{j}_{st}")
                nc.vector.scalar_tensor_tensor(hs2[:, :, :], g2["o"][:, :, :],
                                               1.0, th2[:, :, :], ALU.add, ALU.mult)
                hs2_[st] = hs2
            for st in (0, 1):
                A2, Z2 = attention(j, st, XT[st], hs2_[st], 2, L, UT)
                ft = fpool.tile([128, 4, L], bf16, tag=f"feat{st}", bufs=2,
                                name=f"feat{j}_{st}")
                for zz in range(2):
                    nc.vector.tensor_scalar_max(ft[:, zz, :],
                                                hs2_[st][:, zz, :], 0.0)
                r_matmul(j, st, XN[st], A2, Z2, ft, 2, True, L, UT)
                ft_[st] = ft

            # ---- logits + exp-sum for conversation j (exp table set) ----
            pb = 32 * (j % 3)
            cb = (j // 3) * T_MAX
            pl = gpsum.tile([128, T_MAX], f32, tag="pg", name=f"pl{j}")
            for kt in range(8):
                rhs = ft_[kt // 4][:, kt % 4, :]
                mm(pl[pb:pb + C, 0:L], outw_t[:, kt, :], rhs, kt == 0, kt == 7)
            nc.scalar.activation(lgb[pb:pb + C, cb:cb + L], pl[pb:pb + C, 0:L],
                                 AF.Identity, bias=outb_t[pb:pb + C, 0:1])
            elg = work.tile([71, T_MAX], bf16, tag="elg", bufs=2, name=f"elg{j}")
            nc.scalar.activation(elg[pb:pb + C, 0:L], lgb[pb:pb + C, cb:cb + L],
                                 AF.Exp)
            s1 = spsum.tile([128, T_MAX], f32, tag="psm", name=f"s1_{j}")
            mm(s1[pb:pb + 1, 0:L], ones[pb:pb + C, 0:1], elg[pb:pb + C, 0:L],
               True, True)
            nc.scalar.activation(srows[pb:pb + 1, cb:cb + L], s1[pb:pb + 1, 0:L],
                                 AF.Copy)

        for j in range(NCONV):
            front(j)
            if j == 1:
                for dst, srcap in deferred_dmas:
                    nc.gpsimd.dma_start(out=dst, in_=srcap)
            if j > 0:
                back(j - 1)
        # chunks 0..NCH-2 cover convs 0..14: emitted before back(15) so
        # their log-softmax tail hides under the last conversation's compute
        _tail_chunks(list(range(NCH - 1)))
        back(NCONV - 1)
        _tail_chunks([NCH - 1])



    nc.compile()
    return nc


def _host_prep(inputs):
    """Fold weights, pick the conversation->core assignment, build per-core arrays."""
    x_s = np.asarray(inputs["input"], dtype=np.float32)
    x_p = np.asarray(inputs["speakers"], dtype=np.float32)
    lengths = np.asarray(inputs["utterance_lengths"]).astype(np.int64)
    fc_w = np.asarray(inputs["fc_w"], dtype=np.float32)
    fc_b = np.asarray(inputs["fc_b"], dtype=np.float32)
    out_w = np.asarray(inputs["out_w"], dtype=np.float32)
    out_b = np.asarray(inputs["out_b"], dtype=np.float32)

    per_stream = {}
    any_b1 = False
    any_b2 = False
    for st in ("s", "p"):
        w_ih = np.asarray(inputs[f"w_ih_{st}"], dtype=np.float32)
        w_hh = np.asarray(inputs[f"w_hh_{st}"], dtype=np.float32)
        b_ih = np.asarray(inputs[f"b_ih_{st}"], dtype=np.float32)
        b_hh = np.asarray(inputs[f"b_hh_{st}"], dtype=np.float32)
        W_eff = w_ih @ fc_w                          # [1024, 256]
        bias1 = w_ih @ fc_b + b_ih + b_hh            # [1024]
        sel = np.r_[0:D, 2 * D:4 * D]                # i, g, o rows
        We_sel = np.ascontiguousarray(W_eff[sel].T).astype(BF16)  # [256, 768]
        b1_sel = np.ascontiguousarray(bias1[sel])[None, :].astype(BF16)
        Wh = np.ascontiguousarray((0.5 * (w_ih[:, :D] + w_hh)).T).astype(BF16)
        Wr = np.ascontiguousarray(w_ih[:, D:].T).astype(BF16)     # [256, 1024]
        b2 = np.ascontiguousarray(b_ih + b_hh)[None, :].astype(BF16)
        per_stream[st] = (We_sel, Wh, Wr, b1_sel, b2)
        any_b1 |= bool(np.any(np.asarray(b1_sel, np.float32) != 0.0))
        any_b2 |= bool(np.any(np.asarray(b2, np.float32) != 0.0))

    # out_w columns for the h-halves get the 0.5 compensation (h is stored as 2h)
    ow = out_w.copy()
    ow[:, 0:D] *= 0.5
    ow[:, 2 * D:3 * D] *= 0.5
    outw = np.ascontiguousarray(ow.T).astype(BF16)   # [1024, 7]
    outb = np.zeros((128, 1), dtype=np.float32)      # replicated at rows 32i+c
    for i in range(3):
        outb[32 * i:32 * i + C, 0] = out_b

    # selector: ln-sum at partition 32i -> class rows 32i..32i+6
    sel71 = np.zeros((65, 71), dtype=np.float32)
    for i in range(3):
        sel71[32 * i, 32 * i:32 * i + C] = 1.0

    # conversation -> (core, slot): sort by length desc, round-robin over cores
    order = np.argsort(-lengths, kind="stable")
    assign = {}   # conv -> (core, slot); slot 0 = shortest, last = longest
    for rank, conv in enumerate(order):
        assign[int(conv)] = (rank % NCORE, NCONV - 1 - rank // NCORE)

    order_lens = lengths[order]
    slot_lens = tuple(int(order_lens[8 * (NCONV - 1 - k)])
                      for k in range(NCONV))

    x_s16 = x_s.astype(BF16)
    x_p16 = x_p.astype(BF16)

    in_maps = []
    core_convs = []
    for core in range(NCORE):
        ids = [None] * NCONV
        for conv, (c, s) in assign.items():
            if c == core:
                ids[s] = conv
        core_convs.append(ids)
        mask = np.zeros((128, NCONV * 4), dtype=np.float32)
        for s, conv in enumerate(ids):
            Lc = int(lengths[conv])
            u = np.arange(T_MAX)
            m = np.where(u < Lc, 0.0, MASKV).astype(np.float32)
            mask[:, s * 4:(s + 1) * 4] = m.reshape(4, 128).T
        im = {
            "xs": np.ascontiguousarray(x_s16[:, ids, :]),
            "xp": np.ascontiguousarray(x_p16[:, ids, :]),
            "xst": np.ascontiguousarray(
                x_s16[:, ids, :].transpose(1, 2, 0).reshape(NCONV, 2, 128, T_MAX)),
            "xpt": np.ascontiguousarray(
                x_p16[:, ids, :].transpose(1, 2, 0).reshape(NCONV, 2, 128, T_MAX)),
            "mask": mask,
            "ones_in": np.ones((128, 128), dtype=BF16),
            "sel71": sel71,
            "outw": outw,
            "outb": outb,
        }
        for st in ("s", "p"):
            We_sel, Wh, Wr, b1_sel, b2 = per_stream[st]
            im[f"we_{st}"] = We_sel
            im[f"wh_{st}"] = Wh
            im[f"wr_{st}"] = Wr
            im[f"b1_{st}"] = b1_sel
            im[f"b2_{st}"] = b2
        in_maps.append(im)
    return in_maps, core_convs, lengths, any_b1, any_b2, slot_lens


def _gather(results, core_convs, lengths):
    """results: list (per core) of {'out': [NCONV, C, T_MAX]} -> [sum(len), C]."""
    where = {}
    for core, ids in enumerate(core_convs):
        for slot, conv in enumerate(ids):
            where[conv] = (core, slot)
    chunks = []
    for b in range(BATCH):
        core, slot = where[b]
        L = int(lengths[b])
        chunks.append(np.ascontiguousarray(results[core]["out"][slot, :, :L].T))
    return np.concatenate(chunks, axis=0).astype(np.float32)


def _get_nc(any_b1, any_b2, slot_lens):
    key = (any_b1, any_b2, slot_lens)
    if key not in _BUILD_CACHE:
        _BUILD_CACHE[key] = _build(any_b1, any_b2, slot_lens)
    return _BUILD_CACHE[key]


def kernel(**inputs):
    from concourse import bass_utils
    in_maps, core_convs, lengths, any_b1, any_b2, slot_lens = _host_prep(inputs)
    nc = _get_nc(any_b1, any_b2, slot_lens)
    res = bass_utils.run_bass_kernel_spmd(nc, in_maps, core_ids=list(range(NCORE)))
    return _gather(res.results, core_convs, lengths)
